# revision 38
# baseline (speedup 1.0000x reference)
"""DoubleFeatureTransformerSlice — Trainium2 Bass kernel.

out_s[b, :] = bias + sum_k values_s[b, k] * weight[indices_s[b, k], :]   (s = 0, 1)

Sharding: data-parallel over batch across 8 NeuronCores; weight replicated.
Each core handles 1024 rows of slice0 + 1024 rows of slice1 (16 tiles of 128
samples); per (sample, k) one 1-4 KB weight row is fetched — 65536 random-row
fetches per core.

THE BOTTLENECK (established by A/B this session): every indexed-DMA path on
trn2 goes through SWDGE (gpsimd Q7 software descriptor generation), which
costs ~7-8 ns PER GATHERED ROW regardless of row bytes:
  - indirect_dma_start: ~1.12 us/instruction (128 rows) — f32 572-605 us,
    fp8 572 us: byte-count irrelevant, per-instruction fixed cost rules.
  - dma_gather (1024 rows/call): f32 1054 us (byte-bound at ~260 GB/s),
    fp16 509 us, fp8 528 us (row-bound ~8 ns/row; single_packet=False
    shaves ~8%: fp8 486 us).
  - multi-queue SWDGE: 4 queues x 1024-desc calls RACES (out1 NaN — ring
    overflow: the DynamicDMAScratch splits per queue). 2 queues x 512-desc
    calls (fp8q2) is CORRECT but no faster (589 vs 553 us A/B) — the
    ~7 ns/row is Q7 index-processing, not per-ring drain; queues don't
    parallelize it.
  - batching J>1 rows per indirect DMA via [128, J] offset AP WEDGES the
    device (NRT_EXEC_UNIT_UNRECOVERABLE) — do not use.
So ~65536 rows x ~7 ns ~= 460 us/core is the SWDGE floor; the only partial
escape is overlapping the two SWDGE instruction types' non-Q7 portions.

Modes (same-process interleaved A/B slope, NQ=48, R=1 vs 3 — the reliable
protocol; earlier single-run numbers scattered -25%/+10%):
  fp8s  (SHIPPED) — 556 us, rel err 1.33e-2 (< 2e-2 gate, deterministic
        seed).  dma_gather pulls 8 k-groups x 128 rows (1 KB fp8e3 each)
        per call with single_packet=False; weight cast host-side to fp8e3
        (e3m4, scale 1024; 1/1024 folded into vals).  PE accumulates
        psum += diag(v_k) @ rows_k (fp16 diag x fp8 rhs — mixed-dtype
        matmul verified bit-consistent with the numpy e3m4 simulation);
        DVE builds diags and adds bias.
  fp8h  — hybrid 16 indirect + 2 dma_gather per tile: 622 us in the same
        A/B (the hoped-for indirect/gather overlap does not pay off).
  fp8s16 — fp8s with 2048-row gather calls (gpg=16): correct, ~510-567 us
        across runs — no reliable win over fp8s; kept for reference.
  fp8q2 — 2 SWDGE queues x 512-row calls: correct, 589 us vs fp8s 553 us
        in the same A/B — queue parallelism does not beat the Q7 floor.
  fp16h — fp8h with fp16 weights: rel err 2.9e-4 but 741 us: the 2x bytes
        congest the SDMA side.
  fp8g/fp8i/fp16i/fp16/f32g — single-path variants kept for reference
        (~520-1054 us under the noisy protocol).
  f32  — exact (rel err ~3e-7): previous baseline, 818 us by harness NTFF
        profile (605 us by the previous session's slope).
"""

import numpy as np

MODE = "fp8s"  # which variant kernel() runs (see docstring)

NCORES = 8
B = 8192
K = 32
D = 1024
V = 22528
P = 128
BPC = B // NCORES          # batch rows per core per slice
ROWS = 2 * BPC             # rows per core (slice0 chunk + slice1 chunk)
NTILES = ROWS // P         # 16 tiles of 128 samples
GPG = 8                    # k-values per dma_gather in fp16 mode
NIDX = GPG * P             # num_idxs per dma_gather (1024)
NGATH = NTILES * (K // GPG)  # gathers per core in fp16 mode (64)

_cached = {}
LAST_RESULTS = None        # BassKernelResults of the last run (for harness)


def _build_f32(repeats: int = 1, gath_bufs: int = 32, accp_bufs: int = 6, io_bufs: int = 4, preload_io: bool = True):
    import concourse.bacc as bacc
    import concourse.bass as bass
    import concourse.mybir as mybir
    import concourse.tile as tile

    nc = bacc.Bacc(
        "TRN2",
        target_bir_lowering=False,
        debug=False,
        enable_asserts=False,
        num_devices=NCORES,
    )
    w = nc.dram_tensor("w", [V, D], mybir.dt.float32, kind="ExternalInput")
    idx = nc.dram_tensor("idx", [ROWS, K], mybir.dt.int32, kind="ExternalInput")
    val = nc.dram_tensor("val", [ROWS, K], mybir.dt.float32, kind="ExternalInput")
    bias = nc.dram_tensor("bias_bcast", [P, D], mybir.dt.float32, kind="ExternalInput")
    out = nc.dram_tensor("out", [ROWS, D], mybir.dt.float32, kind="ExternalOutput")

    with tile.TileContext(nc) as tc:
        with (
            tc.tile_pool(name="gath", bufs=gath_bufs) as gpool,
            tc.tile_pool(name="accp", bufs=accp_bufs) as apool,
            tc.tile_pool(name="io", bufs=io_bufs) as iopool,
            tc.tile_pool(name="const", bufs=1) as cpool,
        ):
            bias_t = cpool.tile([P, D], mybir.dt.float32)
            nc.sync.dma_start(bias_t[:], bias[:, :])
            if preload_io:
                # all 16 tiles' indices/values resident up front:
                # idx/val are [ROWS, K] row-major; tile t's rows occupy the
                # contiguous [128, NTILES*K] column band [t*K, (t+1)*K).
                idx_all = cpool.tile([P, NTILES, K], mybir.dt.int32, tag="idxa")
                val_all = cpool.tile([P, NTILES, K], mybir.dt.float32, tag="vala")
                nc.sync.dma_start(idx_all[:], idx[:, :].rearrange("(t p) k -> p t k", p=P))
                nc.sync.dma_start(val_all[:], val[:, :].rearrange("(t p) k -> p t k", p=P))
            for t in range(NTILES * repeats):
                t = t % NTILES
                r0 = t * P
                if preload_io:
                    idx_t = idx_all[:, t]
                    val_t = val_all[:, t]
                else:
                    idx_t = iopool.tile([P, K], mybir.dt.int32, tag="idx")
                    val_t = iopool.tile([P, K], mybir.dt.float32, tag="val")
                    nc.sync.dma_start(idx_t[:], idx[r0 : r0 + P, :])
                    nc.sync.dma_start(val_t[:], val[r0 : r0 + P, :])
                acc = apool.tile([P, D], mybir.dt.float32, tag="acc")
                for k in range(K):
                    g = gpool.tile([P, D], mybir.dt.float32, tag="g")
                    nc.gpsimd.indirect_dma_start(
                        out=g[:],
                        out_offset=None,
                        in_=w[:, :],
                        in_offset=bass.IndirectOffsetOnAxis(
                            ap=idx_t[:, k : k + 1], axis=0
                        ),
                    )
                    nc.vector.scalar_tensor_tensor(
                        out=acc[:],
                        in0=g[:],
                        scalar=val_t[:, k : k + 1],
                        in1=(bias_t[:] if k == 0 else acc[:]),
                        op0=mybir.AluOpType.mult,
                        op1=mybir.AluOpType.add,
                    )
                nc.sync.dma_start(out[r0 : r0 + P, :], acc[:])
    nc.compile()
    return nc


def _build_pe_indirect(repeats: int = 1, wdt: str = "fp16", gath_bufs: int = 32):
    """Per (tile, k) indirect-DMA gather of 128 weight rows (fp16: 2KB,
    fp8e3: 1KB) + PE accumulate psum += diag(v_k) @ rows_k in f32 PSUM.

    idx/val arrive host-packed as [P, NTILES*K] (partition-major) so the
    preload is a single contiguous DMA per tensor — no strided rearrange.
    For fp8e3 the weight is host-scaled by 1024 (absmax 6.8 < 15.5 max) and
    the 1/1024 dequant is folded into the fp16 val (exact: power of two).
    """
    import concourse.bacc as bacc
    import concourse.bass as bass
    import concourse.mybir as mybir
    import concourse.tile as tile
    from concourse.masks import make_identity

    wdtype = mybir.dt.float16 if wdt == "fp16" else mybir.dt.float8e3

    nc = bacc.Bacc(
        "TRN2",
        target_bir_lowering=False,
        debug=False,
        enable_asserts=False,
        num_devices=NCORES,
    )
    w = nc.dram_tensor("w", [V, D], wdtype, kind="ExternalInput")
    idx = nc.dram_tensor("idxp", [P, NTILES * K], mybir.dt.int32, kind="ExternalInput")
    val = nc.dram_tensor("valp", [P, NTILES * K], mybir.dt.float32, kind="ExternalInput")
    bias = nc.dram_tensor("bias_bcast", [P, D], mybir.dt.float32, kind="ExternalInput")
    out = nc.dram_tensor("out", [ROWS, D], mybir.dt.float32, kind="ExternalOutput")

    with tile.TileContext(nc) as tc:
        with (
            tc.tile_pool(name="gath", bufs=gath_bufs) as gpool,
            tc.tile_pool(name="diag", bufs=8) as dpool,
            tc.tile_pool(name="psum", bufs=2, space="PSUM") as ppool,
            tc.tile_pool(name="outs", bufs=3) as opool,
            tc.tile_pool(name="const", bufs=1) as cpool,
        ):
            ident = cpool.tile([P, P], mybir.dt.float16, tag="ident")
            make_identity(nc, ident[:])
            bias_t = cpool.tile([P, D], mybir.dt.float32, tag="bias")
            nc.sync.dma_start(bias_t[:], bias[:, :])
            idx_all = cpool.tile([P, NTILES * K], mybir.dt.int32, tag="idxa")
            val_all = cpool.tile([P, NTILES * K], mybir.dt.float32, tag="vala")
            nc.sync.dma_start(idx_all[:], idx[:, :])
            nc.sync.dma_start(val_all[:], val[:, :])
            for t in range(NTILES * repeats):
                t = t % NTILES
                r0 = t * P
                psum = ppool.tile([P, D], mybir.dt.float32, tag="ps")
                for k in range(K):
                    c = t * K + k
                    g = gpool.tile([P, D], wdtype, tag="g")
                    nc.gpsimd.indirect_dma_start(
                        out=g[:],
                        out_offset=None,
                        in_=w[:, :],
                        in_offset=bass.IndirectOffsetOnAxis(
                            ap=idx_all[:, c : c + 1], axis=0
                        ),
                    )
                    diag = dpool.tile([P, P], mybir.dt.float16, tag="dg")
                    nc.vector.tensor_scalar(
                        out=diag[:],
                        in0=ident[:],
                        scalar1=val_all[:, c : c + 1],
                        scalar2=None,
                        op0=mybir.AluOpType.mult,
                    )
                    first, last = k == 0, k == K - 1
                    nc.tensor.matmul(
                        out=psum[:, 0:512],
                        lhsT=diag[:],
                        rhs=g[:, 0:512],
                        start=first,
                        stop=last,
                    )
                    nc.tensor.matmul(
                        out=psum[:, 512:1024],
                        lhsT=diag[:],
                        rhs=g[:, 512:1024],
                        start=first,
                        stop=last,
                    )
                outt = opool.tile([P, D], mybir.dt.float32, tag="o")
                nc.vector.tensor_tensor(
                    out=outt[:], in0=psum[:], in1=bias_t[:], op=mybir.AluOpType.add
                )
                nc.sync.dma_start(out[r0 : r0 + P, :], outt[:])
    nc.compile()
    return nc


def _build_fp16(
    repeats: int = 1,
    wdt: str = "fp16",
    gath_bufs: int = 3,
    queues: int = 1,
    single_packet: bool = True,
    gpg: int = GPG,
):
    import concourse.bacc as bacc
    import concourse.mybir as mybir
    import concourse.tile as tile
    from concourse.masks import make_identity

    wdtype = mybir.dt.float16 if wdt == "fp16" else mybir.dt.float8e3

    nc = bacc.Bacc(
        "TRN2",
        target_bir_lowering=False,
        debug=False,
        enable_asserts=False,
        num_devices=NCORES,
        num_swdge_queues=queues,
    )
    nidx = gpg * P
    ngath = NTILES * (K // gpg)
    w = nc.dram_tensor("w", [V, D], wdtype, kind="ExternalInput")
    idx16 = nc.dram_tensor(
        "idx16", [P, ngath * (nidx // 16)], mybir.dt.int16, kind="ExternalInput"
    )
    val = nc.dram_tensor("val", [ROWS, K], mybir.dt.float32, kind="ExternalInput")
    bias = nc.dram_tensor("bias_bcast", [P, D], mybir.dt.float32, kind="ExternalInput")
    out = nc.dram_tensor("out", [ROWS, D], mybir.dt.float32, kind="ExternalOutput")

    CPG = nidx // 16  # idx columns per gather (64)

    with tile.TileContext(nc) as tc:
        with (
            tc.tile_pool(name="gath", bufs=gath_bufs) as gpool,
            tc.tile_pool(name="diag", bufs=6) as dpool,
            tc.tile_pool(name="psum", bufs=2, space="PSUM") as ppool,
            tc.tile_pool(name="outs", bufs=3) as opool,
            tc.tile_pool(name="io", bufs=3) as iopool,
            tc.tile_pool(name="const", bufs=1) as cpool,
        ):
            ident = cpool.tile([P, P], mybir.dt.float16, tag="ident")
            make_identity(nc, ident[:])
            bias_t = cpool.tile([P, D], mybir.dt.float32, tag="bias")
            nc.sync.dma_start(bias_t[:], bias[:, :])
            idxs = cpool.tile([P, ngath * CPG], mybir.dt.int16, tag="idxs")
            nc.sync.dma_start(idxs[:], idx16[:, :])
            for t in range(NTILES * repeats):
                t = t % NTILES
                r0 = t * P
                val_t = iopool.tile([P, K], mybir.dt.float32, tag="val")
                nc.sync.dma_start(val_t[:], val[r0 : r0 + P, :])
                psum = ppool.tile([P, D], mybir.dt.float32, tag="ps")
                for gi in range(K // gpg):
                    gid = t * (K // gpg) + gi
                    g = gpool.tile([P, gpg, D], wdtype, tag="g")
                    nc.gpsimd.dma_gather(
                        g[:],
                        w[:, :],
                        idxs[:, gid * CPG : (gid + 1) * CPG],
                        nidx,
                        nidx,
                        D,
                        queue_num=gid % queues,
                        single_packet=single_packet,
                    )
                    for j in range(gpg):
                        k = gi * gpg + j
                        diag = dpool.tile([P, P], mybir.dt.float16, tag="dg")
                        nc.vector.tensor_scalar(
                            out=diag[:],
                            in0=ident[:],
                            scalar1=val_t[:, k : k + 1],
                            scalar2=None,
                            op0=mybir.AluOpType.mult,
                        )
                        first, last = k == 0, k == K - 1
                        nc.tensor.matmul(
                            out=psum[:, 0:512],
                            lhsT=diag[:],
                            rhs=g[:, j, 0:512],
                            start=first,
                            stop=last,
                        )
                        nc.tensor.matmul(
                            out=psum[:, 512:1024],
                            lhsT=diag[:],
                            rhs=g[:, j, 512:1024],
                            start=first,
                            stop=last,
                        )
                outt = opool.tile([P, D], mybir.dt.float32, tag="o")
                nc.vector.tensor_tensor(
                    out=outt[:], in0=psum[:], in1=bias_t[:], op=mybir.AluOpType.add
                )
                nc.sync.dma_start(out[r0 : r0 + P, :], outt[:])
    nc.compile()
    return nc


def _build_f32g(repeats: int = 1):
    """f32 accuracy, but gathers via dma_gather (8 k-groups x 128 rows of
    4 KB per call) instead of 512 single-k indirect DMAs."""
    import concourse.bacc as bacc
    import concourse.mybir as mybir
    import concourse.tile as tile

    nc = bacc.Bacc(
        "TRN2",
        target_bir_lowering=False,
        debug=False,
        enable_asserts=False,
        num_devices=NCORES,
    )
    w = nc.dram_tensor("w", [V, D], mybir.dt.float32, kind="ExternalInput")
    idx16 = nc.dram_tensor(
        "idx16", [P, NGATH * (NIDX // 16)], mybir.dt.int16, kind="ExternalInput"
    )
    val = nc.dram_tensor("val", [ROWS, K], mybir.dt.float32, kind="ExternalInput")
    bias = nc.dram_tensor("bias_bcast", [P, D], mybir.dt.float32, kind="ExternalInput")
    out = nc.dram_tensor("out", [ROWS, D], mybir.dt.float32, kind="ExternalOutput")

    CPG = NIDX // 16

    with tile.TileContext(nc) as tc:
        with (
            tc.tile_pool(name="gath", bufs=3) as gpool,
            tc.tile_pool(name="accp", bufs=3) as apool,
            tc.tile_pool(name="io", bufs=3) as iopool,
            tc.tile_pool(name="const", bufs=1) as cpool,
        ):
            bias_t = cpool.tile([P, D], mybir.dt.float32, tag="bias")
            nc.sync.dma_start(bias_t[:], bias[:, :])
            idxs = cpool.tile([P, NGATH * CPG], mybir.dt.int16, tag="idxs")
            nc.sync.dma_start(idxs[:], idx16[:, :])
            for t in range(NTILES * repeats):
                t = t % NTILES
                r0 = t * P
                val_t = iopool.tile([P, K], mybir.dt.float32, tag="val")
                nc.sync.dma_start(val_t[:], val[r0 : r0 + P, :])
                acc = apool.tile([P, D], mybir.dt.float32, tag="acc")
                for gi in range(K // GPG):
                    gid = t * (K // GPG) + gi
                    g = gpool.tile([P, GPG, D], mybir.dt.float32, tag="g")
                    nc.gpsimd.dma_gather(
                        g[:],
                        w[:, :],
                        idxs[:, gid * CPG : (gid + 1) * CPG],
                        NIDX,
                        NIDX,
                        D,
                    )
                    for j in range(GPG):
                        k = gi * GPG + j
                        nc.vector.scalar_tensor_tensor(
                            out=acc[:],
                            in0=g[:, j, :],
                            scalar=val_t[:, k : k + 1],
                            in1=(bias_t[:] if k == 0 else acc[:]),
                            op0=mybir.AluOpType.mult,
                            op1=mybir.AluOpType.add,
                        )
                nc.sync.dma_start(out[r0 : r0 + P, :], acc[:])
    nc.compile()
    return nc


KI = 16  # fp8h: k's 0..KI-1 via indirect DMA, KI..31 via dma_gather


def _build_fp8h(repeats: int = 1, gath_bufs: int = 6, ind_bufs: int = 24, wdt: str = "fp8e3", ki: int = KI):
    """Hybrid: per tile, KI k's via indirect DMA (gpsimd-engine-bound,
    ~1.1us/instr) overlapped with (K-KI) k's via dma_gather (SDMA-ring-bound,
    ~8.3us/call of 1024 rows).  The two bottlenecks are different resources;
    emission of the gather calls costs the engine only ~1.3us each."""
    import concourse.bacc as bacc
    import concourse.bass as bass
    import concourse.mybir as mybir
    import concourse.tile as tile
    from concourse.masks import make_identity

    wdtype = mybir.dt.float16 if wdt == "fp16" else mybir.dt.float8e3
    KG = K - ki
    GPT = KG // GPG  # gather calls per tile

    nc = bacc.Bacc(
        "TRN2",
        target_bir_lowering=False,
        debug=False,
        enable_asserts=False,
        num_devices=NCORES,
    )
    w = nc.dram_tensor("w", [V, D], wdtype, kind="ExternalInput")
    idx = nc.dram_tensor("idxp", [P, NTILES * ki], mybir.dt.int32, kind="ExternalInput")
    idx16 = nc.dram_tensor(
        "idx16", [P, NTILES * GPT * (NIDX // 16)], mybir.dt.int16, kind="ExternalInput"
    )
    val = nc.dram_tensor("valp", [P, NTILES * K], mybir.dt.float32, kind="ExternalInput")
    bias = nc.dram_tensor("bias_bcast", [P, D], mybir.dt.float32, kind="ExternalInput")
    out = nc.dram_tensor("out", [ROWS, D], mybir.dt.float32, kind="ExternalOutput")

    CPG = NIDX // 16

    with tile.TileContext(nc) as tc:
        with (
            tc.tile_pool(name="gath", bufs=gath_bufs) as gpool,
            tc.tile_pool(name="gind", bufs=ind_bufs) as ipool,
            tc.tile_pool(name="diag", bufs=8) as dpool,
            tc.tile_pool(name="psum", bufs=2, space="PSUM") as ppool,
            tc.tile_pool(name="outs", bufs=3) as opool,
            tc.tile_pool(name="const", bufs=1) as cpool,
        ):
            ident = cpool.tile([P, P], mybir.dt.float16, tag="ident")
            make_identity(nc, ident[:])
            bias_t = cpool.tile([P, D], mybir.dt.float32, tag="bias")
            nc.sync.dma_start(bias_t[:], bias[:, :])
            idx_all = cpool.tile([P, NTILES * ki], mybir.dt.int32, tag="idxa")
            val_all = cpool.tile([P, NTILES * K], mybir.dt.float32, tag="vala")
            idxs16 = cpool.tile(
                [P, NTILES * GPT * CPG], mybir.dt.int16, tag="idx16"
            )
            nc.sync.dma_start(idx_all[:], idx[:, :])
            nc.sync.dma_start(val_all[:], val[:, :])
            nc.sync.dma_start(idxs16[:], idx16[:, :])

            def mm(psum, diag, rhs_ap, k):
                first, last = k == 0, k == K - 1
                nc.tensor.matmul(
                    out=psum[:, 0:512], lhsT=diag[:], rhs=rhs_ap[:, 0:512],
                    start=first, stop=last,
                )
                nc.tensor.matmul(
                    out=psum[:, 512:1024], lhsT=diag[:], rhs=rhs_ap[:, 512:1024],
                    start=first, stop=last,
                )

            for t in range(NTILES * repeats):
                t = t % NTILES
                r0 = t * P
                psum = ppool.tile([P, D], mybir.dt.float32, tag="ps")
                garr = []
                for gi in range(GPT):
                    gid = t * GPT + gi
                    g = gpool.tile([P, GPG, D], wdtype, tag="gg")
                    nc.gpsimd.dma_gather(
                        g[:],
                        w[:, :],
                        idxs16[:, gid * CPG : (gid + 1) * CPG],
                        NIDX,
                        NIDX,
                        D,
                        single_packet=False,
                    )
                    garr.append(g)
                iarr = []
                for k in range(ki):
                    gi2 = ipool.tile([P, D], wdtype, tag="gi")
                    nc.gpsimd.indirect_dma_start(
                        out=gi2[:],
                        out_offset=None,
                        in_=w[:, :],
                        in_offset=bass.IndirectOffsetOnAxis(
                            ap=idx_all[:, t * ki + k : t * ki + k + 1], axis=0
                        ),
                    )
                    iarr.append(gi2)
                for k in range(K):
                    diag = dpool.tile([P, P], mybir.dt.float16, tag="dg")
                    nc.vector.tensor_scalar(
                        out=diag[:],
                        in0=ident[:],
                        scalar1=val_all[:, t * K + k : t * K + k + 1],
                        scalar2=None,
                        op0=mybir.AluOpType.mult,
                    )
                    if k < ki:
                        mm(psum, diag, iarr[k][:], k)
                    else:
                        j = k - ki
                        mm(psum, diag, garr[j // GPG][:, j % GPG], k)
                outt = opool.tile([P, D], mybir.dt.float32, tag="o")
                nc.vector.tensor_tensor(
                    out=outt[:], in0=psum[:], in1=bias_t[:], op=mybir.AluOpType.add
                )
                nc.sync.dma_start(out[r0 : r0 + P, :], outt[:])
    nc.compile()
    return nc


def _wrap_idx16_sub(idx_c: np.ndarray, k0: int) -> np.ndarray:
    """Like _wrap_idx16 but only k's [k0, K) -> [P, NTILES*GPT*CPG] int16."""
    KG = K - k0
    A = idx_c.reshape(NTILES, P, K)[:, :, k0:].reshape(NTILES, P, KG // GPG, GPG)
    cols = []
    for t in range(NTILES):
        for gi in range(KG // GPG):
            flat = A[t, :, gi, :].T.reshape(-1)
            cols.append(flat.reshape(NIDX // 16, 16).T)
    w16 = np.concatenate(cols, axis=1)
    return np.ascontiguousarray(np.tile(w16, (P // 16, 1)).astype(np.int16))


def _build(repeats: int = 1, mode: str | None = None):
    mode = mode or MODE
    if mode == "f32":
        return _build_f32(repeats)
    if mode == "f32g":
        return _build_f32g(repeats)
    if mode == "fp16i":
        return _build_pe_indirect(repeats, wdt="fp16")
    if mode == "fp8i":
        return _build_pe_indirect(repeats, wdt="fp8e3")
    if mode == "fp8g":
        return _build_fp16(repeats, wdt="fp8e3", gath_bufs=6)
    if mode == "fp8q":
        return _build_fp16(repeats, wdt="fp8e3", gath_bufs=8, queues=4)
    if mode == "fp8s":
        return _build_fp16(repeats, wdt="fp8e3", gath_bufs=8, single_packet=False)
    if mode == "fp8s16":
        return _build_fp16(
            repeats, wdt="fp8e3", gath_bufs=4, single_packet=False, gpg=16
        )
    if mode == "fp8q2":
        # 2 SWDGE queues x 512-desc calls: per-ring pressure ratio matches the
        # proven-safe single-queue/1024-desc config (the 4-queue/1024-desc
        # fp8q NaN'd, consistent with ring overflow).
        return _build_fp16(
            repeats, wdt="fp8e3", gath_bufs=12, queues=2,
            single_packet=False, gpg=4,
        )
    if mode.startswith("fp8h"):
        return _build_fp8h(repeats, ki=int(mode[4:]) if len(mode) > 4 else KI)
    if mode == "fp16h":
        return _build_fp8h(repeats, wdt="fp16")
    return _build_fp16(repeats)


FP8_SCALE = 1024.0  # power of two: dequant folded into fp16 vals exactly


def _pack_pm(a: np.ndarray, dtype) -> np.ndarray:
    """[ROWS, K] -> [P, NTILES*K] partition-major: out[p, t*K+k] = a[t*P+p, k]."""
    return np.ascontiguousarray(
        a.reshape(NTILES, P, K).transpose(1, 0, 2).reshape(P, NTILES * K).astype(dtype)
    )


def _wrap_idx16(idx_c: np.ndarray, gpg: int = GPG) -> np.ndarray:
    """[ROWS, K] int -> [P, ngath * nidx/16] int16 in dma_gather's wrap-16
    layout (index i of a gather lives at [i % 16, i // 16]; pattern replicated
    across all 128 partitions)."""
    nidx = gpg * P
    A = idx_c.reshape(NTILES, P, K // gpg, gpg)  # [t, p, gi, j]
    cols = []
    for t in range(NTILES):
        for gi in range(K // gpg):
            flat = A[t, :, gi, :].T.reshape(-1)  # i = j*128 + p
            cols.append(flat.reshape(nidx // 16, 16).T)  # [16, CPG]
    w16 = np.concatenate(cols, axis=1)  # [16, ngath*CPG]
    return np.ascontiguousarray(np.tile(w16, (P // 16, 1)).astype(np.int16))


def prep_in_maps(fi0, fv0, fi1, fv1, weight, bias, mode=None):
    mode = mode or MODE
    b = np.asarray(bias, dtype=np.float32)
    bias_b = np.ascontiguousarray(np.broadcast_to(b[None, :], (P, D)))
    if mode in ("f32", "f32g"):
        w = np.ascontiguousarray(np.asarray(weight, dtype=np.float32))
    elif mode in ("fp8i", "fp8g", "fp8q", "fp8s", "fp8s16", "fp8q2") or mode.startswith("fp8h"):
        import ml_dtypes

        w = np.ascontiguousarray(
            (np.asarray(weight, dtype=np.float32) * FP8_SCALE).astype(
                ml_dtypes.float8_e3m4
            )
        )
    else:
        w = np.ascontiguousarray(np.asarray(weight).astype(np.float16))
    in_maps = []
    for c in range(NCORES):
        sl = slice(c * BPC, (c + 1) * BPC)
        idx_c = np.concatenate([fi0[sl], fi1[sl]], axis=0)
        m = {"w": w, "bias_bcast": bias_b}
        val_c = np.concatenate([fv0[sl], fv1[sl]], axis=0).astype(np.float32)
        if mode.startswith("fp8h") or mode == "fp16h":
            ki = int(mode[4:]) if mode.startswith("fp8h") and len(mode) > 4 else KI
            m["idxp"] = np.ascontiguousarray(
                idx_c.reshape(NTILES, P, K)[:, :, :ki]
                .transpose(1, 0, 2)
                .reshape(P, NTILES * ki)
                .astype(np.int32)
            )
            m["idx16"] = _wrap_idx16_sub(idx_c, ki)
            if mode.startswith("fp8h"):
                val_c = val_c / FP8_SCALE
            m["valp"] = _pack_pm(val_c, np.float32)
        elif mode in ("fp16i", "fp8i"):
            m["idxp"] = _pack_pm(idx_c, np.int32)
            if mode == "fp8i":
                val_c = val_c / FP8_SCALE
            m["valp"] = _pack_pm(val_c, np.float32)
        elif mode == "f32":
            m["idx"] = np.ascontiguousarray(idx_c.astype(np.int32))
            m["val"] = np.ascontiguousarray(val_c)
        else:
            gpg = {"fp8s16": 16, "fp8q2": 4}.get(mode, GPG)
            m["idx16"] = _wrap_idx16(idx_c, gpg)
            if mode in ("fp8g", "fp8q", "fp8s", "fp8s16", "fp8q2"):
                val_c = val_c / FP8_SCALE
            m["val"] = np.ascontiguousarray(val_c)
        in_maps.append(m)
    return in_maps


def kernel(
    feature_indices_0,
    feature_values_0,
    feature_indices_1,
    feature_values_1,
    weight,
    bias,
):
    global LAST_RESULTS
    from concourse.bass_utils import run_bass_kernel_spmd

    if MODE not in _cached:
        _cached[MODE] = _build(mode=MODE)
    nc = _cached[MODE]

    in_maps = prep_in_maps(
        np.asarray(feature_indices_0),
        np.asarray(feature_values_0),
        np.asarray(feature_indices_1),
        np.asarray(feature_values_1),
        weight,
        bias,
        MODE,
    )
    try:
        res = run_bass_kernel_spmd(nc, in_maps, core_ids=list(range(NCORES)))
    except ModuleNotFoundError:
        # BASS_TRACE set but this axon client lacks the NTFF profile hook
        # (antenv.axon_hooks) — rerun with tracing disabled.
        import os

        os.environ["BASS_NEVER_TRACE"] = "1"
        res = run_bass_kernel_spmd(nc, in_maps, core_ids=list(range(NCORES)))
    LAST_RESULTS = res
    outs = [r["out"] for r in res.results]
    out0 = np.concatenate([o[:BPC] for o in outs], axis=0)
    out1 = np.concatenate([o[BPC:] for o in outs], axis=0)
    return (out0, out1)



# revision 39
# speedup vs baseline: 1.1010x; 1.1010x over previous
"""DoubleFeatureTransformerSlice — Trainium2 Bass kernel.

out_s[b, :] = bias + sum_k values_s[b, k] * weight[indices_s[b, k], :]   (s = 0, 1)

Sharding: data-parallel over batch across 8 NeuronCores; weight replicated.
Each core handles 1024 rows of slice0 + 1024 rows of slice1 (16 tiles of 128
samples); per (sample, k) one 1-4 KB weight row is fetched — 65536 random-row
fetches per core.

THE BOTTLENECK (established by A/B this session): every indexed-DMA path on
trn2 goes through SWDGE (gpsimd Q7 software descriptor generation), which
costs ~7-8 ns PER GATHERED ROW regardless of row bytes:
  - indirect_dma_start: ~1.12 us/instruction (128 rows) — f32 572-605 us,
    fp8 572 us: byte-count irrelevant, per-instruction fixed cost rules.
  - dma_gather (1024 rows/call): f32 1054 us (byte-bound at ~260 GB/s),
    fp16 509 us, fp8 528 us (row-bound ~8 ns/row; single_packet=False
    shaves ~8%: fp8 486 us).
  - multi-queue SWDGE: 4 queues x 1024-desc calls RACES (out1 NaN — ring
    overflow: the DynamicDMAScratch splits per queue). 2 queues x 512-desc
    calls (fp8q2) is CORRECT but no faster (589 vs 553 us A/B) — the
    ~7 ns/row is Q7 index-processing, not per-ring drain; queues don't
    parallelize it.
  - batching J>1 rows per indirect DMA via [128, J] offset AP WEDGES the
    device (NRT_EXEC_UNIT_UNRECOVERABLE) — do not use.
So ~65536 rows x ~7 ns ~= 460 us/core is the SWDGE floor; the only partial
escape is overlapping the two SWDGE instruction types' non-Q7 portions.

Modes (same-process interleaved A/B slope, NQ=48, R=1 vs 3 — the reliable
protocol; earlier single-run numbers scattered -25%/+10%):
  fp8s  (SHIPPED) — 556 us, rel err 1.33e-2 (< 2e-2 gate, deterministic
        seed).  dma_gather pulls 8 k-groups x 128 rows (1 KB fp8e3 each)
        per call with single_packet=False; weight cast host-side to fp8e3
        (e3m4, scale 1024; 1/1024 folded into vals).  PE accumulates
        psum += diag(v_k) @ rows_k (fp16 diag x fp8 rhs — mixed-dtype
        matmul verified bit-consistent with the numpy e3m4 simulation);
        DVE builds diags and adds bias.
  fp8h  — hybrid 16 indirect + 2 dma_gather per tile: 622 us in the same
        A/B (the hoped-for indirect/gather overlap does not pay off).
  fp8s16 — fp8s with 2048-row gather calls (gpg=16): correct, ~510-567 us
        across runs — no reliable win over fp8s; kept for reference.
  fp8q2 — 2 SWDGE queues x 512-row calls: correct, 589 us vs fp8s 553 us
        in the same A/B — queue parallelism does not beat the Q7 floor.
        A 15-trial interleaved single-dispatch A/B (the harness metric)
        also ranks fp8s first: medians fp8s 1069 / fp8s16 1146 / fp8q2
        1226 us (incl. ~0.5 ms per-dispatch overhead; distributions wide).
  fp16h — fp8h with fp16 weights: rel err 2.9e-4 but 741 us: the 2x bytes
        congest the SDMA side.
  fp8g/fp8i/fp16i/fp16/f32g — single-path variants kept for reference
        (~520-1054 us under the noisy protocol).
  f32  — exact (rel err ~3e-7): previous baseline, 818 us by harness NTFF
        profile (605 us by the previous session's slope).
"""

import numpy as np

MODE = "fp8s"  # which variant kernel() runs (see docstring)

NCORES = 8
B = 8192
K = 32
D = 1024
V = 22528
P = 128
BPC = B // NCORES          # batch rows per core per slice
ROWS = 2 * BPC             # rows per core (slice0 chunk + slice1 chunk)
NTILES = ROWS // P         # 16 tiles of 128 samples
GPG = 8                    # k-values per dma_gather in fp16 mode
NIDX = GPG * P             # num_idxs per dma_gather (1024)
NGATH = NTILES * (K // GPG)  # gathers per core in fp16 mode (64)

_cached = {}
LAST_RESULTS = None        # BassKernelResults of the last run (for harness)


def _build_f32(repeats: int = 1, gath_bufs: int = 32, accp_bufs: int = 6, io_bufs: int = 4, preload_io: bool = True):
    import concourse.bacc as bacc
    import concourse.bass as bass
    import concourse.mybir as mybir
    import concourse.tile as tile

    nc = bacc.Bacc(
        "TRN2",
        target_bir_lowering=False,
        debug=False,
        enable_asserts=False,
        num_devices=NCORES,
    )
    w = nc.dram_tensor("w", [V, D], mybir.dt.float32, kind="ExternalInput")
    idx = nc.dram_tensor("idx", [ROWS, K], mybir.dt.int32, kind="ExternalInput")
    val = nc.dram_tensor("val", [ROWS, K], mybir.dt.float32, kind="ExternalInput")
    bias = nc.dram_tensor("bias_bcast", [P, D], mybir.dt.float32, kind="ExternalInput")
    out = nc.dram_tensor("out", [ROWS, D], mybir.dt.float32, kind="ExternalOutput")

    with tile.TileContext(nc) as tc:
        with (
            tc.tile_pool(name="gath", bufs=gath_bufs) as gpool,
            tc.tile_pool(name="accp", bufs=accp_bufs) as apool,
            tc.tile_pool(name="io", bufs=io_bufs) as iopool,
            tc.tile_pool(name="const", bufs=1) as cpool,
        ):
            bias_t = cpool.tile([P, D], mybir.dt.float32)
            nc.sync.dma_start(bias_t[:], bias[:, :])
            if preload_io:
                # all 16 tiles' indices/values resident up front:
                # idx/val are [ROWS, K] row-major; tile t's rows occupy the
                # contiguous [128, NTILES*K] column band [t*K, (t+1)*K).
                idx_all = cpool.tile([P, NTILES, K], mybir.dt.int32, tag="idxa")
                val_all = cpool.tile([P, NTILES, K], mybir.dt.float32, tag="vala")
                nc.sync.dma_start(idx_all[:], idx[:, :].rearrange("(t p) k -> p t k", p=P))
                nc.sync.dma_start(val_all[:], val[:, :].rearrange("(t p) k -> p t k", p=P))
            for t in range(NTILES * repeats):
                t = t % NTILES
                r0 = t * P
                if preload_io:
                    idx_t = idx_all[:, t]
                    val_t = val_all[:, t]
                else:
                    idx_t = iopool.tile([P, K], mybir.dt.int32, tag="idx")
                    val_t = iopool.tile([P, K], mybir.dt.float32, tag="val")
                    nc.sync.dma_start(idx_t[:], idx[r0 : r0 + P, :])
                    nc.sync.dma_start(val_t[:], val[r0 : r0 + P, :])
                acc = apool.tile([P, D], mybir.dt.float32, tag="acc")
                for k in range(K):
                    g = gpool.tile([P, D], mybir.dt.float32, tag="g")
                    nc.gpsimd.indirect_dma_start(
                        out=g[:],
                        out_offset=None,
                        in_=w[:, :],
                        in_offset=bass.IndirectOffsetOnAxis(
                            ap=idx_t[:, k : k + 1], axis=0
                        ),
                    )
                    nc.vector.scalar_tensor_tensor(
                        out=acc[:],
                        in0=g[:],
                        scalar=val_t[:, k : k + 1],
                        in1=(bias_t[:] if k == 0 else acc[:]),
                        op0=mybir.AluOpType.mult,
                        op1=mybir.AluOpType.add,
                    )
                nc.sync.dma_start(out[r0 : r0 + P, :], acc[:])
    nc.compile()
    return nc


def _build_pe_indirect(repeats: int = 1, wdt: str = "fp16", gath_bufs: int = 32):
    """Per (tile, k) indirect-DMA gather of 128 weight rows (fp16: 2KB,
    fp8e3: 1KB) + PE accumulate psum += diag(v_k) @ rows_k in f32 PSUM.

    idx/val arrive host-packed as [P, NTILES*K] (partition-major) so the
    preload is a single contiguous DMA per tensor — no strided rearrange.
    For fp8e3 the weight is host-scaled by 1024 (absmax 6.8 < 15.5 max) and
    the 1/1024 dequant is folded into the fp16 val (exact: power of two).
    """
    import concourse.bacc as bacc
    import concourse.bass as bass
    import concourse.mybir as mybir
    import concourse.tile as tile
    from concourse.masks import make_identity

    wdtype = mybir.dt.float16 if wdt == "fp16" else mybir.dt.float8e3

    nc = bacc.Bacc(
        "TRN2",
        target_bir_lowering=False,
        debug=False,
        enable_asserts=False,
        num_devices=NCORES,
    )
    w = nc.dram_tensor("w", [V, D], wdtype, kind="ExternalInput")
    idx = nc.dram_tensor("idxp", [P, NTILES * K], mybir.dt.int32, kind="ExternalInput")
    val = nc.dram_tensor("valp", [P, NTILES * K], mybir.dt.float32, kind="ExternalInput")
    bias = nc.dram_tensor("bias_bcast", [P, D], mybir.dt.float32, kind="ExternalInput")
    out = nc.dram_tensor("out", [ROWS, D], mybir.dt.float32, kind="ExternalOutput")

    with tile.TileContext(nc) as tc:
        with (
            tc.tile_pool(name="gath", bufs=gath_bufs) as gpool,
            tc.tile_pool(name="diag", bufs=8) as dpool,
            tc.tile_pool(name="psum", bufs=2, space="PSUM") as ppool,
            tc.tile_pool(name="outs", bufs=3) as opool,
            tc.tile_pool(name="const", bufs=1) as cpool,
        ):
            ident = cpool.tile([P, P], mybir.dt.float16, tag="ident")
            make_identity(nc, ident[:])
            bias_t = cpool.tile([P, D], mybir.dt.float32, tag="bias")
            nc.sync.dma_start(bias_t[:], bias[:, :])
            idx_all = cpool.tile([P, NTILES * K], mybir.dt.int32, tag="idxa")
            val_all = cpool.tile([P, NTILES * K], mybir.dt.float32, tag="vala")
            nc.sync.dma_start(idx_all[:], idx[:, :])
            nc.sync.dma_start(val_all[:], val[:, :])
            for t in range(NTILES * repeats):
                t = t % NTILES
                r0 = t * P
                psum = ppool.tile([P, D], mybir.dt.float32, tag="ps")
                for k in range(K):
                    c = t * K + k
                    g = gpool.tile([P, D], wdtype, tag="g")
                    nc.gpsimd.indirect_dma_start(
                        out=g[:],
                        out_offset=None,
                        in_=w[:, :],
                        in_offset=bass.IndirectOffsetOnAxis(
                            ap=idx_all[:, c : c + 1], axis=0
                        ),
                    )
                    diag = dpool.tile([P, P], mybir.dt.float16, tag="dg")
                    nc.vector.tensor_scalar(
                        out=diag[:],
                        in0=ident[:],
                        scalar1=val_all[:, c : c + 1],
                        scalar2=None,
                        op0=mybir.AluOpType.mult,
                    )
                    first, last = k == 0, k == K - 1
                    nc.tensor.matmul(
                        out=psum[:, 0:512],
                        lhsT=diag[:],
                        rhs=g[:, 0:512],
                        start=first,
                        stop=last,
                    )
                    nc.tensor.matmul(
                        out=psum[:, 512:1024],
                        lhsT=diag[:],
                        rhs=g[:, 512:1024],
                        start=first,
                        stop=last,
                    )
                outt = opool.tile([P, D], mybir.dt.float32, tag="o")
                nc.vector.tensor_tensor(
                    out=outt[:], in0=psum[:], in1=bias_t[:], op=mybir.AluOpType.add
                )
                nc.sync.dma_start(out[r0 : r0 + P, :], outt[:])
    nc.compile()
    return nc


def _build_fp16(
    repeats: int = 1,
    wdt: str = "fp16",
    gath_bufs: int = 3,
    queues: int = 1,
    single_packet: bool = True,
    gpg: int = GPG,
):
    import concourse.bacc as bacc
    import concourse.mybir as mybir
    import concourse.tile as tile
    from concourse.masks import make_identity

    wdtype = mybir.dt.float16 if wdt == "fp16" else mybir.dt.float8e3

    nc = bacc.Bacc(
        "TRN2",
        target_bir_lowering=False,
        debug=False,
        enable_asserts=False,
        num_devices=NCORES,
        num_swdge_queues=queues,
    )
    nidx = gpg * P
    ngath = NTILES * (K // gpg)
    w = nc.dram_tensor("w", [V, D], wdtype, kind="ExternalInput")
    idx16 = nc.dram_tensor(
        "idx16", [P, ngath * (nidx // 16)], mybir.dt.int16, kind="ExternalInput"
    )
    val = nc.dram_tensor("val", [ROWS, K], mybir.dt.float32, kind="ExternalInput")
    bias = nc.dram_tensor("bias_bcast", [P, D], mybir.dt.float32, kind="ExternalInput")
    out = nc.dram_tensor("out", [ROWS, D], mybir.dt.float32, kind="ExternalOutput")

    CPG = nidx // 16  # idx columns per gather (64)

    with tile.TileContext(nc) as tc:
        with (
            tc.tile_pool(name="gath", bufs=gath_bufs) as gpool,
            tc.tile_pool(name="diag", bufs=6) as dpool,
            tc.tile_pool(name="psum", bufs=2, space="PSUM") as ppool,
            tc.tile_pool(name="outs", bufs=3) as opool,
            tc.tile_pool(name="io", bufs=3) as iopool,
            tc.tile_pool(name="const", bufs=1) as cpool,
        ):
            ident = cpool.tile([P, P], mybir.dt.float16, tag="ident")
            make_identity(nc, ident[:])
            bias_t = cpool.tile([P, D], mybir.dt.float32, tag="bias")
            nc.sync.dma_start(bias_t[:], bias[:, :])
            idxs = cpool.tile([P, ngath * CPG], mybir.dt.int16, tag="idxs")
            nc.sync.dma_start(idxs[:], idx16[:, :])
            for t in range(NTILES * repeats):
                t = t % NTILES
                r0 = t * P
                val_t = iopool.tile([P, K], mybir.dt.float32, tag="val")
                nc.sync.dma_start(val_t[:], val[r0 : r0 + P, :])
                psum = ppool.tile([P, D], mybir.dt.float32, tag="ps")
                for gi in range(K // gpg):
                    gid = t * (K // gpg) + gi
                    g = gpool.tile([P, gpg, D], wdtype, tag="g")
                    nc.gpsimd.dma_gather(
                        g[:],
                        w[:, :],
                        idxs[:, gid * CPG : (gid + 1) * CPG],
                        nidx,
                        nidx,
                        D,
                        queue_num=gid % queues,
                        single_packet=single_packet,
                    )
                    for j in range(gpg):
                        k = gi * gpg + j
                        diag = dpool.tile([P, P], mybir.dt.float16, tag="dg")
                        nc.vector.tensor_scalar(
                            out=diag[:],
                            in0=ident[:],
                            scalar1=val_t[:, k : k + 1],
                            scalar2=None,
                            op0=mybir.AluOpType.mult,
                        )
                        first, last = k == 0, k == K - 1
                        nc.tensor.matmul(
                            out=psum[:, 0:512],
                            lhsT=diag[:],
                            rhs=g[:, j, 0:512],
                            start=first,
                            stop=last,
                        )
                        nc.tensor.matmul(
                            out=psum[:, 512:1024],
                            lhsT=diag[:],
                            rhs=g[:, j, 512:1024],
                            start=first,
                            stop=last,
                        )
                outt = opool.tile([P, D], mybir.dt.float32, tag="o")
                nc.vector.tensor_tensor(
                    out=outt[:], in0=psum[:], in1=bias_t[:], op=mybir.AluOpType.add
                )
                nc.sync.dma_start(out[r0 : r0 + P, :], outt[:])
    nc.compile()
    return nc


def _build_f32g(repeats: int = 1):
    """f32 accuracy, but gathers via dma_gather (8 k-groups x 128 rows of
    4 KB per call) instead of 512 single-k indirect DMAs."""
    import concourse.bacc as bacc
    import concourse.mybir as mybir
    import concourse.tile as tile

    nc = bacc.Bacc(
        "TRN2",
        target_bir_lowering=False,
        debug=False,
        enable_asserts=False,
        num_devices=NCORES,
    )
    w = nc.dram_tensor("w", [V, D], mybir.dt.float32, kind="ExternalInput")
    idx16 = nc.dram_tensor(
        "idx16", [P, NGATH * (NIDX // 16)], mybir.dt.int16, kind="ExternalInput"
    )
    val = nc.dram_tensor("val", [ROWS, K], mybir.dt.float32, kind="ExternalInput")
    bias = nc.dram_tensor("bias_bcast", [P, D], mybir.dt.float32, kind="ExternalInput")
    out = nc.dram_tensor("out", [ROWS, D], mybir.dt.float32, kind="ExternalOutput")

    CPG = NIDX // 16

    with tile.TileContext(nc) as tc:
        with (
            tc.tile_pool(name="gath", bufs=3) as gpool,
            tc.tile_pool(name="accp", bufs=3) as apool,
            tc.tile_pool(name="io", bufs=3) as iopool,
            tc.tile_pool(name="const", bufs=1) as cpool,
        ):
            bias_t = cpool.tile([P, D], mybir.dt.float32, tag="bias")
            nc.sync.dma_start(bias_t[:], bias[:, :])
            idxs = cpool.tile([P, NGATH * CPG], mybir.dt.int16, tag="idxs")
            nc.sync.dma_start(idxs[:], idx16[:, :])
            for t in range(NTILES * repeats):
                t = t % NTILES
                r0 = t * P
                val_t = iopool.tile([P, K], mybir.dt.float32, tag="val")
                nc.sync.dma_start(val_t[:], val[r0 : r0 + P, :])
                acc = apool.tile([P, D], mybir.dt.float32, tag="acc")
                for gi in range(K // GPG):
                    gid = t * (K // GPG) + gi
                    g = gpool.tile([P, GPG, D], mybir.dt.float32, tag="g")
                    nc.gpsimd.dma_gather(
                        g[:],
                        w[:, :],
                        idxs[:, gid * CPG : (gid + 1) * CPG],
                        NIDX,
                        NIDX,
                        D,
                    )
                    for j in range(GPG):
                        k = gi * GPG + j
                        nc.vector.scalar_tensor_tensor(
                            out=acc[:],
                            in0=g[:, j, :],
                            scalar=val_t[:, k : k + 1],
                            in1=(bias_t[:] if k == 0 else acc[:]),
                            op0=mybir.AluOpType.mult,
                            op1=mybir.AluOpType.add,
                        )
                nc.sync.dma_start(out[r0 : r0 + P, :], acc[:])
    nc.compile()
    return nc


KI = 16  # fp8h: k's 0..KI-1 via indirect DMA, KI..31 via dma_gather


def _build_fp8h(repeats: int = 1, gath_bufs: int = 6, ind_bufs: int = 24, wdt: str = "fp8e3", ki: int = KI):
    """Hybrid: per tile, KI k's via indirect DMA (gpsimd-engine-bound,
    ~1.1us/instr) overlapped with (K-KI) k's via dma_gather (SDMA-ring-bound,
    ~8.3us/call of 1024 rows).  The two bottlenecks are different resources;
    emission of the gather calls costs the engine only ~1.3us each."""
    import concourse.bacc as bacc
    import concourse.bass as bass
    import concourse.mybir as mybir
    import concourse.tile as tile
    from concourse.masks import make_identity

    wdtype = mybir.dt.float16 if wdt == "fp16" else mybir.dt.float8e3
    KG = K - ki
    GPT = KG // GPG  # gather calls per tile

    nc = bacc.Bacc(
        "TRN2",
        target_bir_lowering=False,
        debug=False,
        enable_asserts=False,
        num_devices=NCORES,
    )
    w = nc.dram_tensor("w", [V, D], wdtype, kind="ExternalInput")
    idx = nc.dram_tensor("idxp", [P, NTILES * ki], mybir.dt.int32, kind="ExternalInput")
    idx16 = nc.dram_tensor(
        "idx16", [P, NTILES * GPT * (NIDX // 16)], mybir.dt.int16, kind="ExternalInput"
    )
    val = nc.dram_tensor("valp", [P, NTILES * K], mybir.dt.float32, kind="ExternalInput")
    bias = nc.dram_tensor("bias_bcast", [P, D], mybir.dt.float32, kind="ExternalInput")
    out = nc.dram_tensor("out", [ROWS, D], mybir.dt.float32, kind="ExternalOutput")

    CPG = NIDX // 16

    with tile.TileContext(nc) as tc:
        with (
            tc.tile_pool(name="gath", bufs=gath_bufs) as gpool,
            tc.tile_pool(name="gind", bufs=ind_bufs) as ipool,
            tc.tile_pool(name="diag", bufs=8) as dpool,
            tc.tile_pool(name="psum", bufs=2, space="PSUM") as ppool,
            tc.tile_pool(name="outs", bufs=3) as opool,
            tc.tile_pool(name="const", bufs=1) as cpool,
        ):
            ident = cpool.tile([P, P], mybir.dt.float16, tag="ident")
            make_identity(nc, ident[:])
            bias_t = cpool.tile([P, D], mybir.dt.float32, tag="bias")
            nc.sync.dma_start(bias_t[:], bias[:, :])
            idx_all = cpool.tile([P, NTILES * ki], mybir.dt.int32, tag="idxa")
            val_all = cpool.tile([P, NTILES * K], mybir.dt.float32, tag="vala")
            idxs16 = cpool.tile(
                [P, NTILES * GPT * CPG], mybir.dt.int16, tag="idx16"
            )
            nc.sync.dma_start(idx_all[:], idx[:, :])
            nc.sync.dma_start(val_all[:], val[:, :])
            nc.sync.dma_start(idxs16[:], idx16[:, :])

            def mm(psum, diag, rhs_ap, k):
                first, last = k == 0, k == K - 1
                nc.tensor.matmul(
                    out=psum[:, 0:512], lhsT=diag[:], rhs=rhs_ap[:, 0:512],
                    start=first, stop=last,
                )
                nc.tensor.matmul(
                    out=psum[:, 512:1024], lhsT=diag[:], rhs=rhs_ap[:, 512:1024],
                    start=first, stop=last,
                )

            for t in range(NTILES * repeats):
                t = t % NTILES
                r0 = t * P
                psum = ppool.tile([P, D], mybir.dt.float32, tag="ps")
                garr = []
                for gi in range(GPT):
                    gid = t * GPT + gi
                    g = gpool.tile([P, GPG, D], wdtype, tag="gg")
                    nc.gpsimd.dma_gather(
                        g[:],
                        w[:, :],
                        idxs16[:, gid * CPG : (gid + 1) * CPG],
                        NIDX,
                        NIDX,
                        D,
                        single_packet=False,
                    )
                    garr.append(g)
                iarr = []
                for k in range(ki):
                    gi2 = ipool.tile([P, D], wdtype, tag="gi")
                    nc.gpsimd.indirect_dma_start(
                        out=gi2[:],
                        out_offset=None,
                        in_=w[:, :],
                        in_offset=bass.IndirectOffsetOnAxis(
                            ap=idx_all[:, t * ki + k : t * ki + k + 1], axis=0
                        ),
                    )
                    iarr.append(gi2)
                for k in range(K):
                    diag = dpool.tile([P, P], mybir.dt.float16, tag="dg")
                    nc.vector.tensor_scalar(
                        out=diag[:],
                        in0=ident[:],
                        scalar1=val_all[:, t * K + k : t * K + k + 1],
                        scalar2=None,
                        op0=mybir.AluOpType.mult,
                    )
                    if k < ki:
                        mm(psum, diag, iarr[k][:], k)
                    else:
                        j = k - ki
                        mm(psum, diag, garr[j // GPG][:, j % GPG], k)
                outt = opool.tile([P, D], mybir.dt.float32, tag="o")
                nc.vector.tensor_tensor(
                    out=outt[:], in0=psum[:], in1=bias_t[:], op=mybir.AluOpType.add
                )
                nc.sync.dma_start(out[r0 : r0 + P, :], outt[:])
    nc.compile()
    return nc


def _wrap_idx16_sub(idx_c: np.ndarray, k0: int) -> np.ndarray:
    """Like _wrap_idx16 but only k's [k0, K) -> [P, NTILES*GPT*CPG] int16."""
    KG = K - k0
    A = idx_c.reshape(NTILES, P, K)[:, :, k0:].reshape(NTILES, P, KG // GPG, GPG)
    cols = []
    for t in range(NTILES):
        for gi in range(KG // GPG):
            flat = A[t, :, gi, :].T.reshape(-1)
            cols.append(flat.reshape(NIDX // 16, 16).T)
    w16 = np.concatenate(cols, axis=1)
    return np.ascontiguousarray(np.tile(w16, (P // 16, 1)).astype(np.int16))


def _build(repeats: int = 1, mode: str | None = None):
    mode = mode or MODE
    if mode == "f32":
        return _build_f32(repeats)
    if mode == "f32g":
        return _build_f32g(repeats)
    if mode == "fp16i":
        return _build_pe_indirect(repeats, wdt="fp16")
    if mode == "fp8i":
        return _build_pe_indirect(repeats, wdt="fp8e3")
    if mode == "fp8g":
        return _build_fp16(repeats, wdt="fp8e3", gath_bufs=6)
    if mode == "fp8q":
        return _build_fp16(repeats, wdt="fp8e3", gath_bufs=8, queues=4)
    if mode == "fp8s":
        return _build_fp16(repeats, wdt="fp8e3", gath_bufs=8, single_packet=False)
    if mode == "fp8s16":
        return _build_fp16(
            repeats, wdt="fp8e3", gath_bufs=4, single_packet=False, gpg=16
        )
    if mode == "fp8q2":
        # 2 SWDGE queues x 512-desc calls: per-ring pressure ratio matches the
        # proven-safe single-queue/1024-desc config (the 4-queue/1024-desc
        # fp8q NaN'd, consistent with ring overflow).
        return _build_fp16(
            repeats, wdt="fp8e3", gath_bufs=12, queues=2,
            single_packet=False, gpg=4,
        )
    if mode.startswith("fp8h"):
        return _build_fp8h(repeats, ki=int(mode[4:]) if len(mode) > 4 else KI)
    if mode == "fp16h":
        return _build_fp8h(repeats, wdt="fp16")
    return _build_fp16(repeats)


FP8_SCALE = 1024.0  # power of two: dequant folded into fp16 vals exactly


def _pack_pm(a: np.ndarray, dtype) -> np.ndarray:
    """[ROWS, K] -> [P, NTILES*K] partition-major: out[p, t*K+k] = a[t*P+p, k]."""
    return np.ascontiguousarray(
        a.reshape(NTILES, P, K).transpose(1, 0, 2).reshape(P, NTILES * K).astype(dtype)
    )


def _wrap_idx16(idx_c: np.ndarray, gpg: int = GPG) -> np.ndarray:
    """[ROWS, K] int -> [P, ngath * nidx/16] int16 in dma_gather's wrap-16
    layout (index i of a gather lives at [i % 16, i // 16]; pattern replicated
    across all 128 partitions)."""
    nidx = gpg * P
    A = idx_c.reshape(NTILES, P, K // gpg, gpg)  # [t, p, gi, j]
    cols = []
    for t in range(NTILES):
        for gi in range(K // gpg):
            flat = A[t, :, gi, :].T.reshape(-1)  # i = j*128 + p
            cols.append(flat.reshape(nidx // 16, 16).T)  # [16, CPG]
    w16 = np.concatenate(cols, axis=1)  # [16, ngath*CPG]
    return np.ascontiguousarray(np.tile(w16, (P // 16, 1)).astype(np.int16))


def prep_in_maps(fi0, fv0, fi1, fv1, weight, bias, mode=None):
    mode = mode or MODE
    b = np.asarray(bias, dtype=np.float32)
    bias_b = np.ascontiguousarray(np.broadcast_to(b[None, :], (P, D)))
    if mode in ("f32", "f32g"):
        w = np.ascontiguousarray(np.asarray(weight, dtype=np.float32))
    elif mode in ("fp8i", "fp8g", "fp8q", "fp8s", "fp8s16", "fp8q2") or mode.startswith("fp8h"):
        import ml_dtypes

        w = np.ascontiguousarray(
            (np.asarray(weight, dtype=np.float32) * FP8_SCALE).astype(
                ml_dtypes.float8_e3m4
            )
        )
    else:
        w = np.ascontiguousarray(np.asarray(weight).astype(np.float16))
    in_maps = []
    for c in range(NCORES):
        sl = slice(c * BPC, (c + 1) * BPC)
        idx_c = np.concatenate([fi0[sl], fi1[sl]], axis=0)
        m = {"w": w, "bias_bcast": bias_b}
        val_c = np.concatenate([fv0[sl], fv1[sl]], axis=0).astype(np.float32)
        if mode.startswith("fp8h") or mode == "fp16h":
            ki = int(mode[4:]) if mode.startswith("fp8h") and len(mode) > 4 else KI
            m["idxp"] = np.ascontiguousarray(
                idx_c.reshape(NTILES, P, K)[:, :, :ki]
                .transpose(1, 0, 2)
                .reshape(P, NTILES * ki)
                .astype(np.int32)
            )
            m["idx16"] = _wrap_idx16_sub(idx_c, ki)
            if mode.startswith("fp8h"):
                val_c = val_c / FP8_SCALE
            m["valp"] = _pack_pm(val_c, np.float32)
        elif mode in ("fp16i", "fp8i"):
            m["idxp"] = _pack_pm(idx_c, np.int32)
            if mode == "fp8i":
                val_c = val_c / FP8_SCALE
            m["valp"] = _pack_pm(val_c, np.float32)
        elif mode == "f32":
            m["idx"] = np.ascontiguousarray(idx_c.astype(np.int32))
            m["val"] = np.ascontiguousarray(val_c)
        else:
            gpg = {"fp8s16": 16, "fp8q2": 4}.get(mode, GPG)
            m["idx16"] = _wrap_idx16(idx_c, gpg)
            if mode in ("fp8g", "fp8q", "fp8s", "fp8s16", "fp8q2"):
                val_c = val_c / FP8_SCALE
            m["val"] = np.ascontiguousarray(val_c)
        in_maps.append(m)
    return in_maps


def kernel(
    feature_indices_0,
    feature_values_0,
    feature_indices_1,
    feature_values_1,
    weight,
    bias,
):
    global LAST_RESULTS
    from concourse.bass_utils import run_bass_kernel_spmd

    if MODE not in _cached:
        _cached[MODE] = _build(mode=MODE)
    nc = _cached[MODE]

    in_maps = prep_in_maps(
        np.asarray(feature_indices_0),
        np.asarray(feature_values_0),
        np.asarray(feature_indices_1),
        np.asarray(feature_values_1),
        weight,
        bias,
        MODE,
    )
    try:
        res = run_bass_kernel_spmd(nc, in_maps, core_ids=list(range(NCORES)))
    except ModuleNotFoundError:
        # BASS_TRACE set but this axon client lacks the NTFF profile hook
        # (antenv.axon_hooks) — rerun with tracing disabled.
        import os

        os.environ["BASS_NEVER_TRACE"] = "1"
        res = run_bass_kernel_spmd(nc, in_maps, core_ids=list(range(NCORES)))
    LAST_RESULTS = res
    outs = [r["out"] for r in res.results]
    out0 = np.concatenate([o[:BPC] for o in outs], axis=0)
    out1 = np.concatenate([o[BPC:] for o in outs], axis=0)
    return (out0, out1)

